# revision 3
# baseline (speedup 1.0000x reference)
"""Trainium2 Bass kernel for nn_BiLSTMModel (2-layer BiLSTM, B=1024 T=256 D=5 H=64).

Sharding: pure data parallel over batch across 8 cores (128 samples/core).

Transformed cell (validated vs the jax reference): sigmoids for i/f/o and
for g via 2x weight rows (tanh(g) = 2*sigmoid(2g) - 1), cell state c in
fp16, tanh on the Act engine for the output path:
    z   = W_ih@x_t + W_hh@h_{t-1} + b      fp16 matmuls -> fp32 psum
    S   = sigmoid(z)                       3 ACT ops: {i,g}, {f}, {o} --
                                           the {i,g} op leads the chain
    u   = 2*Sg - 1                         DVE TS (4x mode)
    pt  = u * Si                           DVE TT (2x)
    r   = Sf * c_prev                      DVE TT
    c   = pt + r                           DVE TT
    th  = tanh(c)                          ACT (same table as sigmoid)
    h   = th * So                          DVE TT

Structural changes vs baseline:
  * Layer-1 backward scan eliminated: the model uses only out[:, -1, :], and
    h1b(T-1) is the FIRST step of the backward scan = one cell from zero
    state on comb[T-1].  Layer 1 runs forward-only (256 rounds) + 1 cell.
  * Reversed-bwd storage for layer 0: host packs x for the bwd direction
    time-reversed, so both directions write h at the same round-indexed
    column (one DVE op) and the recurrent matmul takes ONE stacked rhs.
  * Block-diagonal W_hh -> 4 recurrent matmuls (K=128,M=128) per round
    instead of 8; x-projection matmuls carry the bias via a ones-row
    (no bias matmul in layer 0).
  * Layer-0 output re-laid into true time order (comb2) via SBUF->SBUF
    DMA on the idle DMA engines, off the critical path.
  * fp16 everywhere outside PSUM; chain elementwise ops are TS/TT forms
    (the cost model gives scalar_tensor_tensor no fp16 speedup).
  * Per-Act-op PSUM/SBUF tiles (the tile framework serializes multiple
    readers/writers of one tile through semaphores).
"""
import os
import numpy as np

import concourse.bacc as bacc
import concourse.bass as bass
import concourse.mybir as mybir
import concourse.tile as tile
from concourse.bass_utils import run_bass_kernel_spmd

H = 64
B = 128          # per-core batch
NCORES = 8
FULL_T = 256
LA = 2           # rounds of x/input-matmul emission lookahead

F16 = mybir.dt.float16
F32 = mybir.dt.float32
AF = mybir.ActivationFunctionType
ALU = mybir.AluOpType

DUMP = False     # debug: add DRAM dumps of comb2/fcin


# ---------------------------------------------------------------- host packing

def _eff_dir(w_ih, w_hh, b_ih, b_hh):
    """Effective weights for the transformed cell (float64 math).
    Gate row order stays PyTorch [i, f, g, o]; g rows 2x so that
    tanh(g) = 2*(sigmoid(2g) - 0.5)."""
    Wi = np.asarray(w_ih, np.float64).copy()
    Wh = np.asarray(w_hh, np.float64).copy()
    b = (np.asarray(b_ih, np.float64) + np.asarray(b_hh, np.float64)).copy()
    g = slice(2 * H, 3 * H)
    Wi[g] *= 2.0
    Wh[g] *= 2.0
    b[g] *= 2.0
    return Wi, Wh, b


def make_core_inputs(inputs, T):
    w = {}
    eff = {}
    for d, suf in (("f", ""), ("b", "r")):
        for l in (0, 1):
            eff[(l, d)] = _eff_dir(inputs[f"w_ih_l{l}{suf}"], inputs[f"w_hh_l{l}{suf}"],
                                   inputs[f"b_ih_l{l}{suf}"], inputs[f"b_hh_l{l}{suf}"])

    # ---- layer 0: block-diag recurrent + x-proj-with-bias weights per gate
    Wi0f, Wh0f, b0f = eff[(0, "f")]
    Wi0b, Wh0b, b0b = eff[(0, "b")]
    wh0f_T, wh0b_T = Wh0f.T, Wh0b.T          # [64, 256]
    wx0f_T, wx0b_T = Wi0f.T, Wi0b.T          # [5, 256]
    for g, gate in enumerate([0, 2, 1, 3]):      # region order i, g, f, o
        gc = slice(gate * H, (gate + 1) * H)
        wbd = np.zeros((128, 128), np.float64)
        wbd[0:64, 0:64] = wh0f_T[:, gc]
        wbd[64:128, 64:128] = wh0b_T[:, gc]
        w[f"wbd0_{g}"] = wbd.astype(np.float16)
        wx = np.zeros((11, 128), np.float64)
        wx[0:5, 0:64] = wx0f_T[:, gc]
        wx[5:10, 64:128] = wx0b_T[:, gc]
        wx[10, 0:64] = b0f[gc]
        wx[10, 64:128] = b0b[gc]
        w[f"wxbd0_{g}"] = wx.astype(np.float16)

    # ---- layer 1 forward scan weights: units-on-partitions, per gate
    # region tiles: a = {i | g}, b = {f | o}  (PyTorch gate idx i=0 f=1 g=2 o=3)
    Wi1f, Wh1f, b1f = eff[(1, "f")]
    wi1f_T, wh1f_T = Wi1f.T, Wh1f.T          # [128, 256], [64, 256]
    Wi1b, _, b1b = eff[(1, "b")]
    wi1b_T = Wi1b.T
    for rk, gate in (("i", 0), ("g", 2), ("f", 1), ("o", 3)):
        gc = slice(gate * H, (gate + 1) * H)
        w[f"wi1_{rk}"] = wi1f_T[:, gc].astype(np.float16)      # [128, 64]
        w[f"whh1_{rk}"] = wh1f_T[:, gc].astype(np.float16)     # [64, 64]
        w[f"wi1b_{rk}"] = wi1b_T[:, gc].astype(np.float16)
    bias1a = np.zeros((2, 64), np.float64)
    bias1a[0] = b1f[0:64]          # i
    bias1a[1] = b1f[128:192]       # g
    bias1bt = np.zeros((2, 64), np.float64)
    bias1bt[0] = b1f[64:128]       # f
    bias1bt[1] = b1f[192:256]      # o
    w["bias1a"] = bias1a.astype(np.float16)
    w["bias1b_t"] = bias1bt.astype(np.float16)
    biasba = np.zeros((2, 64), np.float64)
    biasba[0] = b1b[0:64]
    biasba[1] = b1b[128:192]
    biasbb = np.zeros((2, 64), np.float64)
    biasbb[0] = b1b[64:128]
    biasbb[1] = b1b[192:256]
    w["biasb_a"] = biasba.astype(np.float16)
    w["biasb_b"] = biasbb.astype(np.float16)

    mask2 = np.zeros((2, 2 * B), np.float16)
    mask2[0, 0:B] = 1.0
    mask2[1, B:2 * B] = 1.0
    w["mask2"] = mask2

    fcw = np.asarray(inputs["fc_w"], np.float64).T     # [128, 1]
    w["fcw_top"] = fcw[0:64].astype(np.float16)
    w["fcw_bot"] = fcw[64:128].astype(np.float16)
    w["fcb"] = np.full((B, 1), float(np.asarray(inputs["fc_b"]).reshape(-1)[0]),
                       np.float32)

    x = np.asarray(inputs["x"])

    def core_map(k):
        xc = x[k * B:(k + 1) * B, :T, :]                    # [B, T, 5]
        xT = np.ascontiguousarray(xc.transpose(2, 1, 0))    # [5, T, B]
        xcomb = np.empty((11, T, B), np.float32)
        xcomb[0:5] = xT
        xcomb[5:10] = xT[:, ::-1, :]
        xcomb[10] = 1.0
        return {"xcomb": xcomb.astype(np.float16), **w}

    return core_map


# ---------------------------------------------------------------- device build

def build_nc(T=FULL_T, num_devices=NCORES, repeat=1):
    nc = bacc.Bacc("TRN2", target_bir_lowering=False, debug=False,
                   num_devices=num_devices)
    xcomb_d = nc.dram_tensor("xcomb", [11, T, B], F16, kind="ExternalInput")
    dshapes = {}
    for g in range(4):
        dshapes[f"wbd0_{g}"] = [128, 128]
        dshapes[f"wxbd0_{g}"] = [11, 128]
    for rk in "igfo":
        dshapes[f"wi1_{rk}"] = [128, 64]
        dshapes[f"whh1_{rk}"] = [64, 64]
        dshapes[f"wi1b_{rk}"] = [128, 64]
    dshapes.update({"bias1a": [2, 64], "bias1b_t": [2, 64],
                    "biasb_a": [2, 64], "biasb_b": [2, 64],
                    "mask2": [2, 2 * B],
                    "fcw_top": [64, 1], "fcw_bot": [64, 1]})
    wd = {n: nc.dram_tensor(n, s, F16, kind="ExternalInput")
          for n, s in dshapes.items()}
    fcb_d = nc.dram_tensor("fcb", [B, 1], F32, kind="ExternalInput")
    out_d = nc.dram_tensor("out", [B, 1], F32, kind="ExternalOutput")

    with tile.TileContext(nc) as tc:
        with (
            tc.tile_pool(name="const", bufs=1) as cp,
            tc.tile_pool(name="wk", bufs=3) as wk,
            tc.tile_pool(name="ps", bufs=8, space="PSUM") as pp,
        ):
            xcomb = cp.tile([11, T * B], F16, tag="xcomb")
            nc.sync.dma_start(xcomb[:], xcomb_d[:])
            W = {}
            for n in dshapes:
                W[n] = cp.tile(dshapes[n], F16, tag=n, name=n)[:]
                nc.sync.dma_start(W[n], wd[n][:])
            fcb_s = cp.tile([B, 1], F32, tag="fcb_s")
            nc.sync.dma_start(fcb_s[:], fcb_d[:])

            comb_t = cp.tile([128, 4 * B], F16, tag="comb_t")   # tilde 4-deep ring
            comb2 = cp.tile([128, T * B], F16, tag="comb2")     # true time order
            s_st = [cp.tile([128, B], F16, tag=f"s{p}", name=f"s{p}")
                    for p in (0, 1)]
            s1_st = [cp.tile([64, B], F16, tag=f"s1{p}", name=f"s1{p}")
                     for p in (0, 1)]
            h1_st = [cp.tile([64, B], F16, tag=f"h1{p}", name=f"h1{p}")
                     for p in (0, 1)]
            h1f_last = cp.tile([64, B], F16, tag="h1f_last")
            h1b_last = cp.tile([64, B], F16, tag="h1b_last")

            for _rep in range(repeat):
              # ================= layer 0: both dirs, bwd time-reversed ======
              nc.vector.memset(s_st[1][:], 0.0)
              ps_q = []

              def emit_x0(j):
                  ps2 = [pp.tile([128, 2 * B], F32, tag="ps2", name=f"ps_{j}_{h}")
                         for h in range(2)]
                  rhs = xcomb[:, j * B:(j + 1) * B]
                  for g in range(4):
                      nc.tensor.matmul(ps2[g // 2][:, (g % 2) * B:(g % 2 + 1) * B],
                                       W[f"wxbd0_{g}"], rhs,
                                       start=(g % 2 == 0), stop=(j == 0 and g % 2 == 1),
                                       skip_group_check=True)
                  ps_q.append(ps2)

              for j in range(min(LA + 1, T)):
                  emit_x0(j)
              for j in range(T):
                  par = j % 2
                  q4 = j % 4
                  p4 = (j - 1) % 4
                  ps_ig, ps_fo = ps_q[j]
                  if j > 0:
                      rhs = comb_t[:, p4 * B:(p4 + 1) * B]
                      for g in range(4):
                          nc.tensor.matmul(
                              (ps_ig, ps_fo)[g // 2][:, (g % 2) * B:(g % 2 + 1) * B],
                              W[f"wbd0_{g}"], rhs,
                              start=False, stop=(g % 2 == 1),
                              skip_group_check=True)
                  if j + LA + 1 < T:
                      emit_x0(j + LA + 1)
                  Sig = wk.tile([128, 2 * B], F16, tag="Sig")
                  Sf = wk.tile([128, B], F16, tag="Sf")
                  So = wk.tile([128, B], F16, tag="So")
                  nc.scalar.activation(Sig[:], ps_ig[:], AF.Sigmoid)
                  nc.scalar.activation(Sf[:], ps_fo[:, 0:B], AF.Sigmoid)
                  nc.scalar.activation(So[:], ps_fo[:, B:2 * B], AF.Sigmoid)
                  u = wk.tile([128, B], F16, tag="u")
                  nc.vector.tensor_scalar(u[:], Sig[:, B:2 * B], 0.5, 2.0,
                                          ALU.subtract, ALU.mult)
                  pt = wk.tile([128, B], F16, tag="pt")
                  nc.vector.tensor_tensor(pt[:], u[:], Sig[:, 0:B], ALU.mult)
                  r = wk.tile([128, B], F16, tag="r")
                  nc.vector.tensor_tensor(r[:], Sf[:], s_st[1 - par][:],
                                          ALU.mult)
                  nc.vector.tensor_tensor(s_st[par][:], pt[:], r[:], ALU.add)
                  ch = wk.tile([128, B], F16, tag="ch")
                  nc.scalar.activation(ch[:], s_st[par][:], AF.Tanh)
                  hcol = comb_t[:, q4 * B:(q4 + 1) * B]
                  nc.vector.tensor_tensor(hcol, ch[:], So[:], ALU.mult)
                  # re-lay into true time order via SBUF->SBUF DMA (off path)
                  nc.sync.dma_start(comb2[0:64, j * B:(j + 1) * B], hcol[0:64, :])
                  tb = T - 1 - j
                  nc.sync.dma_start(comb2[64:128, tb * B:(tb + 1) * B],
                                    hcol[64:128, :])

              # ================= layer 1 forward scan =======================
              nc.vector.memset(s1_st[1][:], 0.0)
              ps1_q = []

              def emit_in1(t):
                  pa = pp.tile([128, 2 * B], F32, tag="ps2", name=f"pa{t}")
                  pb = pp.tile([128, 2 * B], F32, tag="ps2", name=f"pb{t}")
                  rhs = comb2[:, t * B:(t + 1) * B]
                  nc.tensor.matmul(pa[0:64, :], W["bias1a"], W["mask2"],
                                   start=True, stop=False, skip_group_check=True)
                  nc.tensor.matmul(pa[0:64, 0:B], W["wi1_i"], rhs,
                                   start=False, stop=False, skip_group_check=True)
                  nc.tensor.matmul(pa[0:64, B:2 * B], W["wi1_g"], rhs,
                                   start=False, stop=(t == 0),
                                   skip_group_check=True)
                  nc.tensor.matmul(pb[0:64, :], W["bias1b_t"], W["mask2"],
                                   start=True, stop=False, skip_group_check=True)
                  nc.tensor.matmul(pb[0:64, 0:B], W["wi1_f"], rhs,
                                   start=False, stop=False, skip_group_check=True)
                  nc.tensor.matmul(pb[0:64, B:2 * B], W["wi1_o"], rhs,
                                   start=False, stop=(t == 0),
                                   skip_group_check=True)
                  ps1_q.append((pa, pb))

              for t in range(min(LA + 1, T)):
                  emit_in1(t)
              for t in range(T):
                  par = t % 2
                  pa, pb = ps1_q[t]
                  if t > 0:
                      rhs = h1_st[1 - par][:]
                      nc.tensor.matmul(pa[0:64, 0:B], W["whh1_i"], rhs,
                                       start=False, stop=False,
                                       skip_group_check=True)
                      nc.tensor.matmul(pa[0:64, B:2 * B], W["whh1_g"], rhs,
                                       start=False, stop=True,
                                       skip_group_check=True)
                      nc.tensor.matmul(pb[0:64, 0:B], W["whh1_f"], rhs,
                                       start=False, stop=False,
                                       skip_group_check=True)
                      nc.tensor.matmul(pb[0:64, B:2 * B], W["whh1_o"], rhs,
                                       start=False, stop=True,
                                       skip_group_check=True)
                  if t + LA + 1 < T:
                      emit_in1(t + LA + 1)
                  S1 = wk.tile([64, 2 * B], F16, tag="S1")
                  Sf1 = wk.tile([64, B], F16, tag="Sf1")
                  So1 = wk.tile([64, B], F16, tag="So1")
                  nc.scalar.activation(S1[:], pa[0:64, :], AF.Sigmoid)
                  nc.scalar.activation(Sf1[:], pb[0:64, 0:B], AF.Sigmoid)
                  nc.scalar.activation(So1[:], pb[0:64, B:2 * B], AF.Sigmoid)
                  ub = wk.tile([64, B], F16, tag="ub")
                  nc.vector.tensor_scalar(ub[:], S1[:, B:2 * B], 0.5, 2.0,
                                          ALU.subtract, ALU.mult)
                  ptb = wk.tile([64, B], F16, tag="ptb")
                  nc.vector.tensor_tensor(ptb[:], ub[:], S1[:, 0:B], ALU.mult)
                  rb = wk.tile([64, B], F16, tag="rb")
                  nc.vector.tensor_tensor(rb[:], Sf1[:], s1_st[1 - par][:],
                                          ALU.mult)
                  nc.vector.tensor_tensor(s1_st[par][:], ptb[:], rb[:], ALU.add)
                  chb = wk.tile([64, B], F16, tag="chb")
                  nc.scalar.activation(chb[:], s1_st[par][:], AF.Tanh)
                  hout = h1f_last[:] if t == T - 1 else h1_st[par][:]
                  nc.vector.tensor_tensor(hout, chb[:], So1[:], ALU.mult)

              # ================= layer 1 backward: single cell at t=T-1 =====
              qa = pp.tile([128, 2 * B], F32, tag="ps2", name="qa")
              qb = pp.tile([128, 2 * B], F32, tag="ps2", name="qb")
              rhsb = comb2[:, (T - 1) * B:T * B]
              nc.tensor.matmul(qa[0:64, :], W["biasb_a"], W["mask2"],
                               start=True, stop=False, skip_group_check=True)
              nc.tensor.matmul(qa[0:64, 0:B], W["wi1b_i"], rhsb,
                               start=False, stop=False, skip_group_check=True)
              nc.tensor.matmul(qa[0:64, B:2 * B], W["wi1b_g"], rhsb,
                               start=False, stop=True, skip_group_check=True)
              nc.tensor.matmul(qb[0:64, :], W["biasb_b"], W["mask2"],
                               start=True, stop=False, skip_group_check=True)
              nc.tensor.matmul(qb[0:64, 0:B], W["wi1b_f"], rhsb,
                               start=False, stop=False, skip_group_check=True)
              nc.tensor.matmul(qb[0:64, B:2 * B], W["wi1b_o"], rhsb,
                               start=False, stop=True, skip_group_check=True)
              Sb = wk.tile([64, 2 * B], F16, tag="S1")
              Sob = wk.tile([64, B], F16, tag="So1")
              nc.scalar.activation(Sb[:], qa[0:64, :], AF.Sigmoid)
              nc.scalar.activation(Sob[:], qb[0:64, B:2 * B], AF.Sigmoid)
              ubb = wk.tile([64, B], F16, tag="ub")
              nc.vector.tensor_scalar(ubb[:], Sb[:, B:2 * B], 0.5, 2.0,
                                      ALU.subtract, ALU.mult)
              sbb = wk.tile([64, B], F16, tag="rb")
              nc.vector.tensor_tensor(sbb[:], ubb[:], Sb[:, 0:B], ALU.mult)
              chbb = wk.tile([64, B], F16, tag="chb")
              nc.scalar.activation(chbb[:], sbb[:], AF.Tanh)
              nc.vector.tensor_tensor(h1b_last[:], chbb[:], Sob[:], ALU.mult)

            # ================= fc =================
            psfull = pp.tile([128, 2 * B], F32, tag="ps2", name="psfc")
            psf = psfull[0:B, 0:1]
            nc.tensor.matmul(psf, h1f_last[:], W["fcw_top"], start=True,
                             stop=False, skip_group_check=True)
            nc.tensor.matmul(psf, h1b_last[:], W["fcw_bot"], start=False,
                             stop=True, skip_group_check=True)
            outs = wk.tile([B, 1], F32, tag="outs")
            nc.vector.tensor_scalar(outs[:], psf, fcb_s[:], None, ALU.add)
            nc.sync.dma_start(out_d[:], outs[:])

            if DUMP:
                comb2_dbg = nc.dram_tensor("comb2_dbg", [128, T * B], F16,
                                           kind="ExternalOutput")
                nc.sync.dma_start(comb2_dbg[:], comb2[:])

    nc.compile()
    return nc


# ---------------------------------------------------------------- entry points

_NC_CACHE = {}


def _get_nc(T=FULL_T):
    if T not in _NC_CACHE:
        _NC_CACHE[T] = build_nc(T)
    return _NC_CACHE[T]


def kernel(**inputs):
    x = np.asarray(inputs["x"])
    T = x.shape[1]
    nc = _get_nc(T)
    core_map = make_core_inputs(inputs, T)
    in_maps = [core_map(k) for k in range(NCORES)]
    res = run_bass_kernel_spmd(nc, in_maps, list(range(NCORES)),
                               trace=bool(os.environ.get("BASS_TRACE_KERNEL")))
    out = np.concatenate([np.asarray(res.results[k]["out"]) for k in range(NCORES)],
                         axis=0)
    kernel.last_results = res
    return out.astype(np.float32)


# revision 6
# speedup vs baseline: 1.9052x; 1.9052x over previous
"""Trainium2 Bass kernel for nn_BiLSTMModel (2-layer BiLSTM, B=1024 T=256 D=5 H=64).

Sharding: pure data parallel over batch across 8 cores (128 samples/core).

Transformed cell (validated vs the jax reference): sigmoids for i/f/o and
for g via 2x weight rows (tanh(g) = 2*sigmoid(2g) - 1), cell state c in
fp16, tanh on the Act engine for the output path:
    z   = W_ih@x_t + W_hh@h_{t-1} + b      fp16 matmuls -> fp32 psum
    S   = sigmoid(z)                       3 ACT ops: {i,g}, {f}, {o} --
                                           the {i,g} op leads the chain
    u   = 2*Sg - 1                         DVE TS (4x mode)
    pt  = u * Si                           DVE TT (2x)
    r   = Sf * c_prev                      DVE TT
    c   = pt + r                           DVE TT
    th  = tanh(c)                          ACT (same table as sigmoid)
    h   = th * So                          DVE TT

Structural changes vs baseline:
  * Layer-1 backward scan eliminated: the model uses only out[:, -1, :], and
    h1b(T-1) is the FIRST step of the backward scan = one cell from zero
    state on comb[T-1].  Layer 1 runs forward-only (256 rounds) + 1 cell.
  * Reversed-bwd storage for layer 0: host packs x for the bwd direction
    time-reversed, so both directions write h at the same round-indexed
    column (one DVE op) and the recurrent matmul takes ONE stacked rhs.
  * Block-diagonal W_hh -> 4 recurrent matmuls (K=128,M=128) per round
    instead of 8; x-projection matmuls carry the bias via a ones-row
    (no bias matmul in layer 0).
  * Layer-0 output re-laid into true time order (comb2) via SBUF->SBUF
    DMA on the idle DMA engines, off the critical path.
  * fp16 everywhere outside PSUM; chain elementwise ops are TS/TT forms
    (the cost model gives scalar_tensor_tensor no fp16 speedup).
  * Per-Act-op PSUM/SBUF tiles (the tile framework serializes multiple
    readers/writers of one tile through semaphores).
"""
import os
import numpy as np

import concourse.bacc as bacc
import concourse.bass as bass
import concourse.mybir as mybir
import concourse.tile as tile
from concourse.bass_utils import run_bass_kernel_spmd

H = 64
B = 128          # per-core batch
NCORES = 8
FULL_T = 256
LA = 2           # rounds of x/input-matmul emission lookahead

F16 = mybir.dt.float16
F32 = mybir.dt.float32
AF = mybir.ActivationFunctionType
ALU = mybir.AluOpType

DUMP = False     # debug: add DRAM dumps of comb2/fcin


# ---------------------------------------------------------------- host packing

def _eff_dir(w_ih, w_hh, b_ih, b_hh):
    """Effective weights for the transformed cell (float64 math).
    Gate row order stays PyTorch [i, f, g, o]; g rows 2x so that
    tanh(g) = 2*(sigmoid(2g) - 0.5)."""
    Wi = np.asarray(w_ih, np.float64).copy()
    Wh = np.asarray(w_hh, np.float64).copy()
    b = (np.asarray(b_ih, np.float64) + np.asarray(b_hh, np.float64)).copy()
    g = slice(2 * H, 3 * H)
    Wi[g] *= 2.0
    Wh[g] *= 2.0
    b[g] *= 2.0
    return Wi, Wh, b


def make_core_inputs(inputs, T):
    w = {}
    eff = {}
    for d, suf in (("f", ""), ("b", "r")):
        for l in (0, 1):
            eff[(l, d)] = _eff_dir(inputs[f"w_ih_l{l}{suf}"], inputs[f"w_hh_l{l}{suf}"],
                                   inputs[f"b_ih_l{l}{suf}"], inputs[f"b_hh_l{l}{suf}"])

    # ---- layer 0: block-diag recurrent + x-proj-with-bias weights per gate
    Wi0f, Wh0f, b0f = eff[(0, "f")]
    Wi0b, Wh0b, b0b = eff[(0, "b")]
    wh0f_T, wh0b_T = Wh0f.T, Wh0b.T          # [64, 256]
    wx0f_T, wx0b_T = Wi0f.T, Wi0b.T          # [5, 256]
    for g, gate in enumerate([0, 2, 1, 3]):      # region order i, g, f, o
        gc = slice(gate * H, (gate + 1) * H)
        wbd = np.zeros((128, 128), np.float64)
        wbd[0:64, 0:64] = wh0f_T[:, gc]
        wbd[64:128, 64:128] = wh0b_T[:, gc]
        w[f"wbd0_{g}"] = wbd.astype(np.float16)
        wx = np.zeros((11, 128), np.float64)
        wx[0:5, 0:64] = wx0f_T[:, gc]
        wx[5:10, 64:128] = wx0b_T[:, gc]
        wx[10, 0:64] = b0f[gc]
        wx[10, 64:128] = b0b[gc]
        w[f"wxbd0_{g}"] = wx.astype(np.float16)

    # ---- layer 1 forward scan weights: units-on-partitions, per gate
    # region tiles: a = {i | g}, b = {f | o}  (PyTorch gate idx i=0 f=1 g=2 o=3)
    Wi1f, Wh1f, b1f = eff[(1, "f")]
    wi1f_T, wh1f_T = Wi1f.T, Wh1f.T          # [128, 256], [64, 256]
    Wi1b, _, b1b = eff[(1, "b")]
    wi1b_T = Wi1b.T
    for rk, gate in (("i", 0), ("g", 2), ("f", 1), ("o", 3)):
        gc = slice(gate * H, (gate + 1) * H)
        w[f"wi1_{rk}"] = wi1f_T[:, gc].astype(np.float16)      # [128, 64]
        whb = np.zeros((65, 64), np.float64)                   # bias rides row 64
        whb[0:64] = wh1f_T[:, gc]
        whb[64] = b1f[gc]
        w[f"whh1_{rk}"] = whb.astype(np.float16)               # [65, 64]
        w[f"wi1b_{rk}"] = wi1b_T[:, gc].astype(np.float16)
    bias1a = np.zeros((2, 64), np.float64)
    bias1a[0] = b1f[0:64]          # i
    bias1a[1] = b1f[128:192]       # g
    bias1bt = np.zeros((2, 64), np.float64)
    bias1bt[0] = b1f[64:128]       # f
    bias1bt[1] = b1f[192:256]      # o
    w["bias1a"] = bias1a.astype(np.float16)
    w["bias1b_t"] = bias1bt.astype(np.float16)
    biasba = np.zeros((2, 64), np.float64)
    biasba[0] = b1b[0:64]
    biasba[1] = b1b[128:192]
    biasbb = np.zeros((2, 64), np.float64)
    biasbb[0] = b1b[64:128]
    biasbb[1] = b1b[192:256]
    w["biasb_a"] = biasba.astype(np.float16)
    w["biasb_b"] = biasbb.astype(np.float16)

    mask2 = np.zeros((2, 2 * B), np.float16)
    mask2[0, 0:B] = 1.0
    mask2[1, B:2 * B] = 1.0
    w["mask2"] = mask2

    fcw = np.asarray(inputs["fc_w"], np.float64).T     # [128, 1]
    w["fcw_top"] = fcw[0:64].astype(np.float16)
    w["fcw_bot"] = fcw[64:128].astype(np.float16)
    w["fcb"] = np.full((B, 1), float(np.asarray(inputs["fc_b"]).reshape(-1)[0]),
                       np.float32)

    x = np.asarray(inputs["x"])

    def core_map(k):
        xc = x[k * B:(k + 1) * B, :T, :]                    # [B, T, 5]
        xT = np.ascontiguousarray(xc.transpose(2, 1, 0))    # [5, T, B]
        xcomb = np.empty((11, T, B), np.float32)
        xcomb[0:5] = xT
        xcomb[5:10] = xT[:, ::-1, :]
        xcomb[10] = 1.0
        return {"xcomb": xcomb.astype(np.float16), **w}

    return core_map


# ---------------------------------------------------------------- device build

def build_nc(T=FULL_T, num_devices=NCORES, repeat=1):
    nc = bacc.Bacc("TRN2", target_bir_lowering=False, debug=False,
                   num_devices=num_devices)
    xcomb_d = nc.dram_tensor("xcomb", [11, T, B], F16, kind="ExternalInput")
    dshapes = {}
    for g in range(4):
        dshapes[f"wbd0_{g}"] = [128, 128]
        dshapes[f"wxbd0_{g}"] = [11, 128]
    for rk in "igfo":
        dshapes[f"wi1_{rk}"] = [128, 64]
        dshapes[f"whh1_{rk}"] = [65, 64]
        dshapes[f"wi1b_{rk}"] = [128, 64]
    dshapes.update({"bias1a": [2, 64], "bias1b_t": [2, 64],
                    "biasb_a": [2, 64], "biasb_b": [2, 64],
                    "mask2": [2, 2 * B],
                    "fcw_top": [64, 1], "fcw_bot": [64, 1]})
    wd = {n: nc.dram_tensor(n, s, F16, kind="ExternalInput")
          for n, s in dshapes.items()}
    fcb_d = nc.dram_tensor("fcb", [B, 1], F32, kind="ExternalInput")
    out_d = nc.dram_tensor("out", [B, 1], F32, kind="ExternalOutput")

    with tile.TileContext(nc) as tc:
        with (
            tc.tile_pool(name="const", bufs=1) as cp,
            tc.tile_pool(name="wk", bufs=3) as wk,
            tc.tile_pool(name="ps", bufs=8, space="PSUM") as pp,
        ):
            xcomb = cp.tile([11, T * B], F16, tag="xcomb")
            nc.sync.dma_start(xcomb[:], xcomb_d[:])
            W = {}
            for n in dshapes:
                W[n] = cp.tile(dshapes[n], F16, tag=n, name=n)[:]
                nc.sync.dma_start(W[n], wd[n][:])
            fcb_s = cp.tile([B, 1], F32, tag="fcb_s")
            nc.sync.dma_start(fcb_s[:], fcb_d[:])

            comb_t = cp.tile([128, 8 * B], F16, tag="comb_t")   # tilde 8-deep ring
            comb2 = cp.tile([128, T * B], F16, tag="comb2")     # true time order
            s_st = [cp.tile([128, B], F16, tag=f"s{p}", name=f"s{p}")
                    for p in (0, 1)]
            s1_st = [cp.tile([64, B], F16, tag=f"s1{p}", name=f"s1{p}")
                     for p in (0, 1)]
            h1_st = [cp.tile([65, B], F16, tag=f"h1{p}", name=f"h1{p}")
                     for p in (0, 1)]
            h1f_last = cp.tile([64, B], F16, tag="h1f_last")
            h1b_last = cp.tile([64, B], F16, tag="h1b_last")

            for _rep in range(repeat):
              # ================= layer 0: both dirs, bwd time-reversed ======
              nc.vector.memset(s_st[1][:], 0.0)
              ps_q = []

              def emit_x0(j):
                  ps2 = [pp.tile([128, 2 * B], F32, tag="ps2", name=f"ps_{j}_{h}")
                         for h in range(2)]
                  rhs = xcomb[:, j * B:(j + 1) * B]
                  for g in range(4):
                      nc.tensor.matmul(ps2[g // 2][:, (g % 2) * B:(g % 2 + 1) * B],
                                       W[f"wxbd0_{g}"], rhs,
                                       start=(g % 2 == 0), stop=(j == 0 and g % 2 == 1),
                                       skip_group_check=True)
                  ps_q.append(ps2)

              for j in range(min(LA + 1, T)):
                  emit_x0(j)
              for j in range(T):
                  par = j % 2
                  q4 = j % 8
                  p4 = (j - 1) % 8
                  ps_ig, ps_fo = ps_q[j]
                  if j > 0:
                      rhs = comb_t[:, p4 * B:(p4 + 1) * B]
                      for g in range(4):
                          nc.tensor.matmul(
                              (ps_ig, ps_fo)[g // 2][:, (g % 2) * B:(g % 2 + 1) * B],
                              W[f"wbd0_{g}"], rhs,
                              start=False, stop=(g % 2 == 1),
                              skip_group_check=True)
                  if j + LA + 1 < T:
                      emit_x0(j + LA + 1)
                  Sig = wk.tile([128, 2 * B], F16, tag="Sig")
                  Sf = wk.tile([128, B], F16, tag="Sf")
                  So = wk.tile([128, B], F16, tag="So")
                  nc.scalar.activation(Sig[:], ps_ig[:], AF.Sigmoid)
                  nc.scalar.activation(Sf[:], ps_fo[:, 0:B], AF.Sigmoid)
                  nc.scalar.activation(So[:], ps_fo[:, B:2 * B], AF.Sigmoid)
                  pt = wk.tile([128, B], F16, tag="pt")
                  nc.vector.scalar_tensor_tensor(pt[:], Sig[:, B:2 * B], 0.5,
                                                 Sig[:, 0:B], ALU.subtract,
                                                 ALU.mult)
                  r = wk.tile([128, B], F16, tag="r")
                  nc.vector.tensor_tensor(r[:], Sf[:], s_st[1 - par][:],
                                          ALU.mult)
                  nc.vector.scalar_tensor_tensor(s_st[par][:], pt[:], 2.0, r[:],
                                                 ALU.mult, ALU.add)
                  ch = wk.tile([128, B], F16, tag="ch")
                  nc.scalar.activation(ch[:], s_st[par][:], AF.Tanh)
                  hcol = comb_t[:, q4 * B:(q4 + 1) * B]
                  nc.vector.tensor_tensor(hcol, ch[:], So[:], ALU.mult)
                  # re-lay into true time order via SBUF->SBUF DMA (off path);
                  # fwd half batched every 4 rounds (ring cols are contiguous)
                  if j % 4 == 3 or j == T - 1:
                      lo = (j // 4) * 4
                      c0 = lo % 8
                      n = j - lo + 1
                      nc.sync.dma_start(comb2[0:64, lo * B:(j + 1) * B],
                                        comb_t[0:64, c0 * B:(c0 + n) * B])
                  tb = T - 1 - j
                  nc.sync.dma_start(comb2[64:128, tb * B:(tb + 1) * B],
                                    hcol[64:128, :])

              # ================= layer 1 forward scan =======================
              nc.vector.memset(s1_st[1][:], 0.0)
              nc.vector.memset(h1_st[0][:], 0.0)
              nc.vector.memset(h1_st[1][:], 0.0)
              nc.vector.memset(h1_st[0][64:65, :], 1.0)
              nc.vector.memset(h1_st[1][64:65, :], 1.0)
              ps1_q = []

              def emit_in1(t):
                  pa = pp.tile([128, 2 * B], F32, tag="ps2", name=f"pa{t}")
                  pb = pp.tile([128, 2 * B], F32, tag="ps2", name=f"pb{t}")
                  rhs = comb2[:, t * B:(t + 1) * B]
                  nc.tensor.matmul(pa[0:64, 0:B], W["wi1_i"], rhs,
                                   start=True, stop=False, skip_group_check=True)
                  nc.tensor.matmul(pa[0:64, B:2 * B], W["wi1_g"], rhs,
                                   start=False, stop=False, skip_group_check=True)
                  nc.tensor.matmul(pb[0:64, 0:B], W["wi1_f"], rhs,
                                   start=True, stop=False, skip_group_check=True)
                  nc.tensor.matmul(pb[0:64, B:2 * B], W["wi1_o"], rhs,
                                   start=False, stop=False, skip_group_check=True)
                  ps1_q.append((pa, pb))

              for t in range(min(LA + 1, T)):
                  emit_in1(t)
              for t in range(T):
                  par = t % 2
                  pa, pb = ps1_q[t]
                  if True:
                      rhs = h1_st[1 - par][:]
                      nc.tensor.matmul(pa[0:64, 0:B], W["whh1_i"], rhs,
                                       start=False, stop=False,
                                       skip_group_check=True)
                      nc.tensor.matmul(pa[0:64, B:2 * B], W["whh1_g"], rhs,
                                       start=False, stop=True,
                                       skip_group_check=True)
                      nc.tensor.matmul(pb[0:64, 0:B], W["whh1_f"], rhs,
                                       start=False, stop=False,
                                       skip_group_check=True)
                      nc.tensor.matmul(pb[0:64, B:2 * B], W["whh1_o"], rhs,
                                       start=False, stop=True,
                                       skip_group_check=True)
                  if t + LA + 1 < T:
                      emit_in1(t + LA + 1)
                  S1 = wk.tile([64, 2 * B], F16, tag="S1")
                  Sf1 = wk.tile([64, B], F16, tag="Sf1")
                  So1 = wk.tile([64, B], F16, tag="So1")
                  nc.scalar.activation(S1[:], pa[0:64, :], AF.Sigmoid)
                  nc.scalar.activation(Sf1[:], pb[0:64, 0:B], AF.Sigmoid)
                  nc.scalar.activation(So1[:], pb[0:64, B:2 * B], AF.Sigmoid)
                  ptb = wk.tile([64, B], F16, tag="ptb")
                  nc.vector.scalar_tensor_tensor(ptb[:], S1[:, B:2 * B], 0.5,
                                                 S1[:, 0:B], ALU.subtract,
                                                 ALU.mult)
                  rb = wk.tile([64, B], F16, tag="rb")
                  nc.vector.tensor_tensor(rb[:], Sf1[:], s1_st[1 - par][:],
                                          ALU.mult)
                  nc.vector.scalar_tensor_tensor(s1_st[par][:], ptb[:], 2.0,
                                                 rb[:], ALU.mult, ALU.add)
                  chb = wk.tile([64, B], F16, tag="chb")
                  nc.scalar.activation(chb[:], s1_st[par][:], AF.Tanh)
                  hout = h1f_last[:] if t == T - 1 else h1_st[par][0:64, :]
                  nc.vector.tensor_tensor(hout, chb[:], So1[:], ALU.mult)

              # ================= layer 1 backward: single cell at t=T-1 =====
              qa = pp.tile([128, 2 * B], F32, tag="ps2", name="qa")
              qb = pp.tile([128, 2 * B], F32, tag="ps2", name="qb")
              rhsb = comb2[:, (T - 1) * B:T * B]
              nc.tensor.matmul(qa[0:64, :], W["biasb_a"], W["mask2"],
                               start=True, stop=False, skip_group_check=True)
              nc.tensor.matmul(qa[0:64, 0:B], W["wi1b_i"], rhsb,
                               start=False, stop=False, skip_group_check=True)
              nc.tensor.matmul(qa[0:64, B:2 * B], W["wi1b_g"], rhsb,
                               start=False, stop=True, skip_group_check=True)
              nc.tensor.matmul(qb[0:64, :], W["biasb_b"], W["mask2"],
                               start=True, stop=False, skip_group_check=True)
              nc.tensor.matmul(qb[0:64, 0:B], W["wi1b_f"], rhsb,
                               start=False, stop=False, skip_group_check=True)
              nc.tensor.matmul(qb[0:64, B:2 * B], W["wi1b_o"], rhsb,
                               start=False, stop=True, skip_group_check=True)
              Sb = wk.tile([64, 2 * B], F16, tag="S1")
              Sob = wk.tile([64, B], F16, tag="So1")
              nc.scalar.activation(Sb[:], qa[0:64, :], AF.Sigmoid)
              nc.scalar.activation(Sob[:], qb[0:64, B:2 * B], AF.Sigmoid)
              ubb = wk.tile([64, B], F16, tag="ub")
              nc.vector.tensor_scalar(ubb[:], Sb[:, B:2 * B], 0.5, 2.0,
                                      ALU.subtract, ALU.mult)
              sbb = wk.tile([64, B], F16, tag="rb")
              nc.vector.tensor_tensor(sbb[:], ubb[:], Sb[:, 0:B], ALU.mult)
              chbb = wk.tile([64, B], F16, tag="chb")
              nc.scalar.activation(chbb[:], sbb[:], AF.Tanh)
              nc.vector.tensor_tensor(h1b_last[:], chbb[:], Sob[:], ALU.mult)

            # ================= fc =================
            psfull = pp.tile([128, 2 * B], F32, tag="ps2", name="psfc")
            psf = psfull[0:B, 0:1]
            nc.tensor.matmul(psf, h1f_last[:], W["fcw_top"], start=True,
                             stop=False, skip_group_check=True)
            nc.tensor.matmul(psf, h1b_last[:], W["fcw_bot"], start=False,
                             stop=True, skip_group_check=True)
            outs = wk.tile([B, 1], F32, tag="outs")
            nc.vector.tensor_scalar(outs[:], psf, fcb_s[:], None, ALU.add)
            nc.sync.dma_start(out_d[:], outs[:])

            if DUMP:
                comb2_dbg = nc.dram_tensor("comb2_dbg", [128, T * B], F16,
                                           kind="ExternalOutput")
                nc.sync.dma_start(comb2_dbg[:], comb2[:])

    nc.compile()
    return nc


# ---------------------------------------------------------------- entry points

_NC_CACHE = {}


def _get_nc(T=FULL_T):
    if T not in _NC_CACHE:
        _NC_CACHE[T] = build_nc(T)
    return _NC_CACHE[T]


def kernel(**inputs):
    x = np.asarray(inputs["x"])
    T = x.shape[1]
    nc = _get_nc(T)
    core_map = make_core_inputs(inputs, T)
    in_maps = [core_map(k) for k in range(NCORES)]
    res = run_bass_kernel_spmd(nc, in_maps, list(range(NCORES)),
                               trace=bool(os.environ.get("BASS_TRACE_KERNEL")))
    out = np.concatenate([np.asarray(res.results[k]["out"]) for k in range(NCORES)],
                         axis=0)
    kernel.last_results = res
    return out.astype(np.float32)
